# revision 6
# baseline (speedup 1.0000x reference)
"""Multi-head attention TRN2 Bass kernel (B=4, S=1024, D=512, H=8, per-head dim 512).

Sharding: 8 cores = (batch, head-group) grid: core c handles batch c//2 and
heads (c%2)*4 .. (c%2)*4+3. Each core computes q/k/v projections for its
heads (feature-major via DMA-transposed x), masked softmax (fp32, exact
reference semantics incl. fully-masked rows), the attention output slice,
and a partial y.T = Wo_slice.T @ ctx.T which the host sums across the two
head-group cores per batch. All matmuls in bf16 with fp32 PSUM accumulation;
softmax math in fp32.
"""

import numpy as np
import ml_dtypes

import concourse.bass as bass
import concourse.mybir as mybir
import concourse.tile as tile
from concourse import bacc, bass_utils
from concourse.alu_op_type import AluOpType

F32 = mybir.dt.float32
BF16 = mybir.dt.bfloat16
I8 = mybir.dt.int8

B, S, D, H = 4, 1024, 512, 8
HPC = 4                      # heads per core
NEG = float(np.float32(-1.0e15))
S_SCALE = float(1.0 / np.sqrt(np.float32(D)))

_CACHE = {}


def build():
    nc = bacc.Bacc("TRN2", target_bir_lowering=False, debug=False, num_devices=8)

    XQ = nc.dram_tensor("XQ", [S, D], BF16, kind="ExternalInput").ap()
    XK = nc.dram_tensor("XK", [S, D], BF16, kind="ExternalInput").ap()
    XV = nc.dram_tensor("XV", [S, D], BF16, kind="ExternalInput").ap()
    M8 = nc.dram_tensor("M8", [S, S], I8, kind="ExternalInput").ap()
    WQ = nc.dram_tensor("WQ", [D, HPC * D], BF16, kind="ExternalInput").ap()
    WK = nc.dram_tensor("WK", [D, HPC * D], BF16, kind="ExternalInput").ap()
    WV = nc.dram_tensor("WV", [D, HPC * D], BF16, kind="ExternalInput").ap()
    WO = nc.dram_tensor("WO", [HPC * D, D], BF16, kind="ExternalInput").ap()
    BQ = nc.dram_tensor("BQ", [1, HPC * D], BF16, kind="ExternalInput").ap()
    BK = nc.dram_tensor("BK", [1, HPC * D], BF16, kind="ExternalInput").ap()
    BV = nc.dram_tensor("BV", [1, HPC * D], BF16, kind="ExternalInput").ap()

    ATTN = nc.dram_tensor("ATTN", [HPC, S, S], F32, kind="ExternalOutput").ap()
    YT = nc.dram_tensor("YT", [D, S], F32, kind="ExternalOutput").ap()

    NT = S // 128            # 8 token tiles
    NF = D // 128            # 4 feature chunks

    with tile.TileContext(nc) as tc:
        with (
            tc.tile_pool(name="pc", bufs=1) as pc,
            tc.tile_pool(name="px", bufs=1) as px,
            tc.tile_pool(name="pstage", bufs=3) as pstage,
            tc.tile_pool(name="pw", bufs=1) as pw,
            tc.tile_pool(name="pqkv", bufs=2) as pqkv,
            tc.tile_pool(name="psm", bufs=3) as psm,
            tc.tile_pool(name="psmall", bufs=6) as psmall,
            tc.tile_pool(name="pet", bufs=1) as pet,
            tc.tile_pool(name="pps_proj", bufs=2, space="PSUM") as pps_proj,
            tc.tile_pool(name="pps_log", bufs=2, space="PSUM") as pps_log,
            tc.tile_pool(name="pps_ctx", bufs=2, space="PSUM") as pps_ctx,
            tc.tile_pool(name="pps_y", bufs=2, space="PSUM") as pps_y,
        ):
            ones512 = pc.tile([1, 512], BF16, tag="ones512")
            nc.vector.memset(ones512[:], 1.0)

            # ---- load + transpose x inputs to feature-major xT (bf16) ----
            xqT = [px.tile([128, S], BF16, tag=f"xqT{f}", name=f"xqT{f}") for f in range(NF)]
            xkT = [px.tile([128, S], BF16, tag=f"xkT{f}", name=f"xkT{f}") for f in range(NF)]
            xvT = [px.tile([128, S], BF16, tag=f"xvT{f}", name=f"xvT{f}") for f in range(NF)]
            for xin, xT in ((XQ, xqT), (XK, xkT), (XV, xvT)):
                for t in range(NT):
                    st = pstage.tile([128, D], BF16, tag="stage")
                    nc.sync.dma_start(st[:], xin[t * 128:(t + 1) * 128, :])
                    for f in range(NF):
                        nc.sync.dma_start(
                            xT[f][:, t * 128:(t + 1) * 128],
                            st[:, f * 128:(f + 1) * 128], transpose=True)

            # ---- masks (int8, 0/1/2), one [128,S] tile per sq-tile ----
            msk = [px.tile([128, S], I8, tag=f"m{i}", name=f"m{i}") for i in range(NT)]
            for i in range(NT):
                nc.sync.dma_start(msk[i][:], M8[i * 128:(i + 1) * 128, :])

            # ---- y.T accumulator (fp32) ----
            yacc = [px.tile([128, S], F32, tag=f"yacc{m}", name=f"yacc{m}") for m in range(NF)]
            for m in range(NF):
                nc.vector.memset(yacc[m][:], 0.0)

            for h in range(HPC):
                hs = h * D       # column offset of this head in the packed weights

                # ---- per-head weight tiles ----
                wq = [pw.tile([128, D], BF16, tag=f"wq{k}", name=f"wq{k}_h{h}") for k in range(NF)]
                wk = [pw.tile([128, D], BF16, tag=f"wk{k}", name=f"wk{k}_h{h}") for k in range(NF)]
                wv = [pw.tile([128, D], BF16, tag=f"wv{k}", name=f"wv{k}_h{h}") for k in range(NF)]
                wo = [pw.tile([128, D], BF16, tag=f"wo{k}", name=f"wo{k}_h{h}") for k in range(NF)]
                for k in range(NF):
                    nc.sync.dma_start(wq[k][:], WQ[k * 128:(k + 1) * 128, hs:hs + D])
                    nc.sync.dma_start(wk[k][:], WK[k * 128:(k + 1) * 128, hs:hs + D])
                    nc.sync.dma_start(wv[k][:], WV[k * 128:(k + 1) * 128, hs:hs + D])
                    nc.sync.dma_start(wo[k][:], WO[hs + k * 128:hs + (k + 1) * 128, :])
                bq = pw.tile([1, D], BF16, tag="bq")
                nc.sync.dma_start(bq[:], BQ[:, hs:hs + D])
                bk = pw.tile([1, D], BF16, tag="bk")
                nc.sync.dma_start(bk[:], BK[:, hs:hs + D])
                bv = pw.tile([1, D], BF16, tag="bv")
                nc.sync.dma_start(bv[:], BV[:, hs:hs + D])

                # ---- projections ----
                # q,k feature-major: [feat-tile 128, tokens]; q pre-scaled by 1/sqrt(D)
                qT = [pqkv.tile([128, S], BF16, tag=f"qT{m}", name=f"qT{m}_h{h}") for m in range(NF)]
                kT = [pqkv.tile([128, S], BF16, tag=f"kT{m}", name=f"kT{m}_h{h}") for m in range(NF)]
                for dst, w, b, xT, scale in (
                    (qT, wq, bq, xqT, S_SCALE),
                    (kT, wk, bk, xkT, 1.0),
                ):
                    for m in range(NF):
                        for n in range(S // 512):
                            ps = pps_proj.tile([128, 512], F32, tag="proj")
                            for kc in range(NF):
                                nc.tensor.matmul(
                                    ps[:], w[kc][:, m * 128:(m + 1) * 128],
                                    xT[kc][:, n * 512:(n + 1) * 512],
                                    start=(kc == 0), stop=False)
                            nc.tensor.matmul(
                                ps[:], b[:, m * 128:(m + 1) * 128], ones512[:],
                                start=False, stop=True)
                            nc.scalar.activation(
                                dst[m][:, n * 512:(n + 1) * 512], ps[:],
                                mybir.ActivationFunctionType.Copy, scale=scale)

                # v token-major: [tok-tile 128, dv 512]
                v_sb = [pqkv.tile([128, D], BF16, tag=f"v{t}", name=f"v{t}_h{h}") for t in range(NT)]
                for t in range(NT):
                    ps = pps_proj.tile([128, 512], F32, tag="proj")
                    for kc in range(NF):
                        nc.tensor.matmul(
                            ps[:], xvT[kc][:, t * 128:(t + 1) * 128], wv[kc][:],
                            start=(kc == 0), stop=False)
                    nc.tensor.matmul(ps[:], ones512[:, 0:128], bv[:],
                                     start=False, stop=True)
                    nc.scalar.activation(v_sb[t][:], ps[:],
                                         mybir.ActivationFunctionType.Copy)

                # ---- attention: logits -> softmax -> attn out + transposed E ----
                ET = [pet.tile([128, S], BF16, tag=f"ET{j}", name=f"ET{j}_h{h}") for j in range(NT)]
                for i in range(NT):
                    ps_pair = []
                    for nk in range(S // 512):
                        ps = pps_log.tile([128, 512], F32, tag="log")
                        for kc in range(NF):
                            nc.tensor.matmul(
                                ps[:], qT[kc][:, i * 128:(i + 1) * 128],
                                kT[kc][:, nk * 512:(nk + 1) * 512],
                                start=(kc == 0), stop=(kc == NF - 1))
                        ps_pair.append(ps)
                    # X = mask * NEG + logits  (fp32)
                    X = psm.tile([128, S], F32, tag="X")
                    for nk in range(S // 512):
                        nc.vector.scalar_tensor_tensor(
                            X[:, nk * 512:(nk + 1) * 512],
                            msk[i][:, nk * 512:(nk + 1) * 512], NEG,
                            ps_pair[nk][:], op0=AluOpType.mult, op1=AluOpType.add)
                    negmax = psmall.tile([128, 1], F32, tag="negmax")
                    nc.vector.tensor_reduce(
                        negmax[:], X[:], axis=mybir.AxisListType.X,
                        op=AluOpType.max, negate=True)
                    attnf = psm.tile([128, S], F32, tag="attnf")
                    rowsum = psmall.tile([128, 1], F32, tag="rowsum")
                    nc.scalar.activation(
                        attnf[:], X[:], mybir.ActivationFunctionType.Exp,
                        bias=negmax[:], accum_out=rowsum[:])
                    recip = psmall.tile([128, 1], F32, tag="recip")
                    nc.vector.reciprocal(recip[:], rowsum[:])
                    nc.vector.tensor_scalar(attnf[:], attnf[:], recip[:], None,
                                            op0=AluOpType.mult)
                    nc.sync.dma_start(ATTN[h, i * 128:(i + 1) * 128, :], attnf[:])
                    attnb = psm.tile([128, S], BF16, tag="attnb")
                    nc.gpsimd.tensor_copy(attnb[:], attnf[:])
                    for j in range(NT):
                        nc.sync.dma_start(
                            ET[j][:, i * 128:(i + 1) * 128],
                            attnb[:, j * 128:(j + 1) * 128], transpose=True)

                # ---- ctx.T = v.T @ attn.T : [dv-tile 128, sq] ----
                ctxT = [pqkv.tile([128, S], BF16, tag=f"ctxT{m}", name=f"ctxT{m}_h{h}") for m in range(NF)]
                for m in range(NF):
                    for n in range(S // 512):
                        ps = pps_ctx.tile([128, 512], F32, tag="ctx")
                        for skc in range(NT):
                            nc.tensor.matmul(
                                ps[:], v_sb[skc][:, m * 128:(m + 1) * 128],
                                ET[skc][:, n * 512:(n + 1) * 512],
                                start=(skc == 0), stop=(skc == NT - 1))
                        nc.scalar.activation(
                            ctxT[m][:, n * 512:(n + 1) * 512], ps[:],
                            mybir.ActivationFunctionType.Copy)

                # ---- partial out-proj: yacc += Wo_head.T-chunks @ ctx.T ----
                for m in range(NF):
                    for n in range(S // 512):
                        ps = pps_y.tile([128, 512], F32, tag="y")
                        for kc in range(NF):
                            nc.tensor.matmul(
                                ps[:], wo[kc][:, m * 128:(m + 1) * 128],
                                ctxT[kc][:, n * 512:(n + 1) * 512],
                                start=(kc == 0), stop=(kc == NF - 1))
                        nc.vector.tensor_tensor(
                            yacc[m][:, n * 512:(n + 1) * 512],
                            yacc[m][:, n * 512:(n + 1) * 512], ps[:],
                            op=AluOpType.add)

            for m in range(NF):
                nc.sync.dma_start(YT[m * 128:(m + 1) * 128, :], yacc[m][:])

    nc.compile()
    return nc


def _get_nc():
    if "nc" not in _CACHE:
        _CACHE["nc"] = build()
    return _CACHE["nc"]


def prepare_in_maps(query, keys, values, pad_mask, look_ahead_mask,
                    Wq, bq, Wk, bk, Wv, bv, Wo, bo):
    query = np.asarray(query)
    keys = np.asarray(keys)
    values = np.asarray(values)
    m8 = (np.asarray(pad_mask) + np.asarray(look_ahead_mask)).astype(np.int8)

    bf = ml_dtypes.bfloat16
    q16 = query.astype(bf)
    k16 = keys.astype(bf)
    v16 = values.astype(bf)
    Wq16 = np.asarray(Wq).astype(bf)
    Wk16 = np.asarray(Wk).astype(bf)
    Wv16 = np.asarray(Wv).astype(bf)
    Wo16 = np.asarray(Wo).astype(bf)
    bq16 = np.asarray(bq).astype(bf).reshape(1, -1)
    bk16 = np.asarray(bk).astype(bf).reshape(1, -1)
    bv16 = np.asarray(bv).astype(bf).reshape(1, -1)

    in_maps = []
    for c in range(8):
        b, hg = c // 2, c % 2
        cs = slice(hg * HPC * D, (hg + 1) * HPC * D)
        in_maps.append({
            "XQ": q16[b], "XK": k16[b], "XV": v16[b],
            "M8": m8[b],
            "WQ": np.ascontiguousarray(Wq16[:, cs]),
            "WK": np.ascontiguousarray(Wk16[:, cs]),
            "WV": np.ascontiguousarray(Wv16[:, cs]),
            "WO": np.ascontiguousarray(Wo16[cs, :]),
            "BQ": np.ascontiguousarray(bq16[:, cs]),
            "BK": np.ascontiguousarray(bk16[:, cs]),
            "BV": np.ascontiguousarray(bv16[:, cs]),
        })
    return in_maps


def postprocess(res, bo):
    out = np.empty((B, S, D), np.float32)
    attn = np.empty((H, B, S, S), np.float32)
    for c in range(8):
        b, hg = c // 2, c % 2
        r = res.results[c]
        attn[hg * HPC:(hg + 1) * HPC, b] = r["ATTN"]
        if hg == 0:
            out[b] = r["YT"].T
        else:
            out[b] += r["YT"].T
    out += np.asarray(bo).astype(np.float32)
    return out, attn


def kernel(query, keys, values, pad_mask, look_ahead_mask,
           Wq, bq, Wk, bk, Wv, bv, Wo, bo):
    in_maps = prepare_in_maps(query, keys, values, pad_mask, look_ahead_mask,
                              Wq, bq, Wk, bk, Wv, bv, Wo, bo)
    nc = _get_nc()
    res = bass_utils.run_bass_kernel_spmd(nc, in_maps, list(range(8)))
    return postprocess(res, bo)


# revision 7
# speedup vs baseline: 2.1895x; 2.1895x over previous
"""Multi-head attention TRN2 Bass kernel (B=4, S=1024, D=512, H=8, per-head dim 512).

Sharding: 8 cores = (batch, head-group) grid: core c handles batch c//2 and
heads (c%2)*4 .. (c%2)*4+3. Each core computes q/k/v projections for its
heads (feature-major via DMA-transposed x), masked softmax (fp32, exact
reference semantics incl. fully-masked rows), the attention output slice
(bf16, host casts to f32), and a partial y.T = Wo_slice.T @ ctx.T which the
host sums across the two head-group cores per batch. All matmuls bf16 with
fp32 PSUM accumulation; softmax math fp32. attn.T for the A@V matmul comes
from big [S,128]->[128,S] DMA xbar transposes reading back the attn DRAM
output (shadow-memory RAW deps order the readback after the writes).
"""

import numpy as np
import ml_dtypes

import concourse.bass as bass
import concourse.mybir as mybir
import concourse.tile as tile
from concourse import bacc, bass_utils
from concourse.alu_op_type import AluOpType

F32 = mybir.dt.float32
BF16 = mybir.dt.bfloat16
I8 = mybir.dt.int8

B, S, D, H = 4, 1024, 512, 8
HPC = 4                      # heads per core
NEG = float(np.float32(-1.0e15))
S_SCALE = float(1.0 / np.sqrt(np.float32(D)))

_CACHE = {}


def build():
    nc = bacc.Bacc("TRN2", target_bir_lowering=False, debug=False, num_devices=8)

    XQ = nc.dram_tensor("XQ", [S, D], BF16, kind="ExternalInput").ap()
    XK = nc.dram_tensor("XK", [S, D], BF16, kind="ExternalInput").ap()
    XV = nc.dram_tensor("XV", [S, D], BF16, kind="ExternalInput").ap()
    M8 = nc.dram_tensor("M8", [S, S], I8, kind="ExternalInput").ap()
    WQ = nc.dram_tensor("WQ", [D, HPC * D], BF16, kind="ExternalInput").ap()
    WK = nc.dram_tensor("WK", [D, HPC * D], BF16, kind="ExternalInput").ap()
    WV = nc.dram_tensor("WV", [D, HPC * D], BF16, kind="ExternalInput").ap()
    WO = nc.dram_tensor("WO", [HPC * D, D], BF16, kind="ExternalInput").ap()
    BQ = nc.dram_tensor("BQ", [1, HPC * D], BF16, kind="ExternalInput").ap()
    BK = nc.dram_tensor("BK", [1, HPC * D], BF16, kind="ExternalInput").ap()
    BV = nc.dram_tensor("BV", [1, HPC * D], BF16, kind="ExternalInput").ap()

    ATTN = nc.dram_tensor("ATTN", [HPC, S, S], BF16, kind="ExternalOutput").ap()
    YT = nc.dram_tensor("YT", [D, S], F32, kind="ExternalOutput").ap()

    NT = S // 128            # 8 token tiles
    NF = D // 128            # 4 feature chunks

    with tile.TileContext(nc) as tc:
        with (
            tc.tile_pool(name="pc", bufs=1) as pc,
            tc.tile_pool(name="px", bufs=1) as px,
            tc.tile_pool(name="pw", bufs=2) as pw,
            tc.tile_pool(name="pqkv", bufs=2) as pqkv,
            tc.tile_pool(name="psm", bufs=2) as psm,
            tc.tile_pool(name="psmall", bufs=6) as psmall,
            tc.tile_pool(name="pet", bufs=2) as pet,
            tc.tile_pool(name="pps_proj", bufs=2, space="PSUM") as pps_proj,
            tc.tile_pool(name="pps_log", bufs=2, space="PSUM") as pps_log,
            tc.tile_pool(name="pps_ctx", bufs=2, space="PSUM") as pps_ctx,
            tc.tile_pool(name="pps_y", bufs=2, space="PSUM") as pps_y,
        ):
            ones512 = pc.tile([1, 512], BF16, tag="ones512")
            nc.vector.memset(ones512[:], 1.0)

            # ---- feature-major x.T via big DRAM->SBUF xbar transposes ----
            xqT = [px.tile([128, S], BF16, tag=f"xqT{f}", name=f"xqT{f}") for f in range(NF)]
            xkT = [px.tile([128, S], BF16, tag=f"xkT{f}", name=f"xkT{f}") for f in range(NF)]
            xvT = [px.tile([128, S], BF16, tag=f"xvT{f}", name=f"xvT{f}") for f in range(NF)]
            for xin, xT in ((XQ, xqT), (XK, xkT), (XV, xvT)):
                for f in range(NF):
                    nc.sync.dma_start(xT[f][:], xin[:, f * 128:(f + 1) * 128],
                                      transpose=True)

            # ---- masks (int8, 0/1/2), one [128,S] tile per sq-tile ----
            msk = [px.tile([128, S], I8, tag=f"m{i}", name=f"m{i}") for i in range(NT)]
            for i in range(NT):
                nc.sync.dma_start(msk[i][:], M8[i * 128:(i + 1) * 128, :])

            # ---- y.T accumulator (fp32) ----
            yacc = [px.tile([128, S], F32, tag=f"yacc{m}", name=f"yacc{m}") for m in range(NF)]
            for m in range(NF):
                nc.vector.memset(yacc[m][:], 0.0)

            for h in range(HPC):
                hs = h * D

                # ---- per-head weights, one batched 3D-AP DMA each ----
                wq = pw.tile([128, NF, D], BF16, tag="wq", name=f"wq_h{h}")
                wk = pw.tile([128, NF, D], BF16, tag="wk", name=f"wk_h{h}")
                wv = pw.tile([128, NF, D], BF16, tag="wv", name=f"wv_h{h}")
                wo = pw.tile([128, NF, D], BF16, tag="wo", name=f"wo_h{h}")
                nc.sync.dma_start(
                    wq[:], WQ[:, hs:hs + D].rearrange("(k p) m -> p k m", p=128))
                nc.sync.dma_start(
                    wk[:], WK[:, hs:hs + D].rearrange("(k p) m -> p k m", p=128))
                nc.sync.dma_start(
                    wv[:], WV[:, hs:hs + D].rearrange("(k p) m -> p k m", p=128))
                nc.sync.dma_start(
                    wo[:], WO[hs:hs + D, :].rearrange("(k p) m -> p k m", p=128))
                bq = pw.tile([1, D], BF16, tag="bq", name=f"bq_h{h}")
                nc.sync.dma_start(bq[:], BQ[:, hs:hs + D])
                bk = pw.tile([1, D], BF16, tag="bk", name=f"bk_h{h}")
                nc.sync.dma_start(bk[:], BK[:, hs:hs + D])
                bv = pw.tile([1, D], BF16, tag="bv", name=f"bv_h{h}")
                nc.sync.dma_start(bv[:], BV[:, hs:hs + D])

                # ---- projections ----
                # q,k feature-major: [feat-tile 128, tokens]; q pre-scaled by 1/sqrt(D)
                qT = [pqkv.tile([128, S], BF16, tag=f"qT{m}", name=f"qT{m}_h{h}") for m in range(NF)]
                kT = [pqkv.tile([128, S], BF16, tag=f"kT{m}", name=f"kT{m}_h{h}") for m in range(NF)]
                for dst, w, b, xT, scale, use_act in (
                    (qT, wq, bq, xqT, S_SCALE, True),
                    (kT, wk, bk, xkT, 1.0, False),
                ):
                    for m in range(NF):
                        for n in range(S // 512):
                            ps = pps_proj.tile([128, 512], F32, tag="proj")
                            for kc in range(NF):
                                nc.tensor.matmul(
                                    ps[:], w[:, kc, m * 128:(m + 1) * 128],
                                    xT[kc][:, n * 512:(n + 1) * 512],
                                    start=(kc == 0), stop=False)
                            nc.tensor.matmul(
                                ps[:], b[:, m * 128:(m + 1) * 128], ones512[:],
                                start=False, stop=True)
                            if use_act:
                                nc.scalar.activation(
                                    dst[m][:, n * 512:(n + 1) * 512], ps[:],
                                    mybir.ActivationFunctionType.Copy, scale=scale)
                            else:
                                nc.vector.tensor_copy(
                                    dst[m][:, n * 512:(n + 1) * 512], ps[:])

                # v token-major: [tok-tile 128, dv 512]
                v_sb = [pqkv.tile([128, D], BF16, tag=f"v{t}", name=f"v{t}_h{h}") for t in range(NT)]
                for t in range(NT):
                    ps = pps_proj.tile([128, 512], F32, tag="proj")
                    for kc in range(NF):
                        nc.tensor.matmul(
                            ps[:], xvT[kc][:, t * 128:(t + 1) * 128], wv[:, kc, :],
                            start=(kc == 0), stop=False)
                    nc.tensor.matmul(ps[:], ones512[:, 0:128], bv[:],
                                     start=False, stop=True)
                    nc.vector.tensor_copy(v_sb[t][:], ps[:])

                # ---- attention: logits -> softmax -> attn out (bf16) ----
                for i in range(NT):
                    ps_pair = []
                    for nk in range(S // 512):
                        ps = pps_log.tile([128, 512], F32, tag="log")
                        for kc in range(NF):
                            nc.tensor.matmul(
                                ps[:], qT[kc][:, i * 128:(i + 1) * 128],
                                kT[kc][:, nk * 512:(nk + 1) * 512],
                                start=(kc == 0), stop=(kc == NF - 1))
                        ps_pair.append(ps)
                    X = psm.tile([128, S], F32, tag="X")
                    for nk in range(S // 512):
                        nc.vector.scalar_tensor_tensor(
                            X[:, nk * 512:(nk + 1) * 512],
                            msk[i][:, nk * 512:(nk + 1) * 512], NEG,
                            ps_pair[nk][:], op0=AluOpType.mult, op1=AluOpType.add)
                    negmax = psmall.tile([128, 1], F32, tag="negmax")
                    nc.vector.tensor_reduce(
                        negmax[:], X[:], axis=mybir.AxisListType.X,
                        op=AluOpType.max, negate=True)
                    E = psm.tile([128, S], F32, tag="E")
                    rowsum = psmall.tile([128, 1], F32, tag="rowsum")
                    nc.scalar.activation(
                        E[:], X[:], mybir.ActivationFunctionType.Exp,
                        bias=negmax[:], accum_out=rowsum[:])
                    recip = psmall.tile([128, 1], F32, tag="recip")
                    nc.vector.reciprocal(recip[:], rowsum[:])
                    attnb = psm.tile([128, S], BF16, tag="attnb")
                    nc.scalar.activation(
                        attnb[:], E[:], mybir.ActivationFunctionType.Copy,
                        scale=recip[:])
                    nc.sync.dma_start(ATTN[h, i * 128:(i + 1) * 128, :], attnb[:])

                # ---- attn.T tiles via big DRAM->SBUF xbar transposes ----
                ET = [pet.tile([128, S], BF16, tag=f"ET{j}", name=f"ET{j}_h{h}") for j in range(NT)]
                for j in range(NT):
                    nc.sync.dma_start(ET[j][:], ATTN[h, :, j * 128:(j + 1) * 128],
                                      transpose=True)

                # ---- ctx.T = v.T @ attn.T : [dv-tile 128, sq] ----
                ctxT = [pqkv.tile([128, S], BF16, tag=f"ctxT{m}", name=f"ctxT{m}_h{h}") for m in range(NF)]
                for m in range(NF):
                    for n in range(S // 512):
                        ps = pps_ctx.tile([128, 512], F32, tag="ctx")
                        for skc in range(NT):
                            nc.tensor.matmul(
                                ps[:], v_sb[skc][:, m * 128:(m + 1) * 128],
                                ET[skc][:, n * 512:(n + 1) * 512],
                                start=(skc == 0), stop=(skc == NT - 1))
                        nc.scalar.activation(
                            ctxT[m][:, n * 512:(n + 1) * 512], ps[:],
                            mybir.ActivationFunctionType.Copy)

                # ---- partial out-proj: yacc += Wo_head chunks @ ctx.T ----
                for m in range(NF):
                    for n in range(S // 512):
                        ps = pps_y.tile([128, 512], F32, tag="y")
                        for kc in range(NF):
                            nc.tensor.matmul(
                                ps[:], wo[:, kc, m * 128:(m + 1) * 128],
                                ctxT[kc][:, n * 512:(n + 1) * 512],
                                start=(kc == 0), stop=(kc == NF - 1))
                        nc.vector.tensor_tensor(
                            yacc[m][:, n * 512:(n + 1) * 512],
                            yacc[m][:, n * 512:(n + 1) * 512], ps[:],
                            op=AluOpType.add)

            for m in range(NF):
                nc.sync.dma_start(YT[m * 128:(m + 1) * 128, :], yacc[m][:])

    nc.compile()
    return nc


def _get_nc():
    if "nc" not in _CACHE:
        _CACHE["nc"] = build()
    return _CACHE["nc"]


def prepare_in_maps(query, keys, values, pad_mask, look_ahead_mask,
                    Wq, bq, Wk, bk, Wv, bv, Wo, bo):
    query = np.asarray(query)
    keys = np.asarray(keys)
    values = np.asarray(values)
    m8 = (np.asarray(pad_mask) + np.asarray(look_ahead_mask)).astype(np.int8)

    bf = ml_dtypes.bfloat16
    q16 = query.astype(bf)
    k16 = keys.astype(bf)
    v16 = values.astype(bf)
    Wq16 = np.asarray(Wq).astype(bf)
    Wk16 = np.asarray(Wk).astype(bf)
    Wv16 = np.asarray(Wv).astype(bf)
    Wo16 = np.asarray(Wo).astype(bf)
    bq16 = np.asarray(bq).astype(bf).reshape(1, -1)
    bk16 = np.asarray(bk).astype(bf).reshape(1, -1)
    bv16 = np.asarray(bv).astype(bf).reshape(1, -1)

    in_maps = []
    for c in range(8):
        b, hg = c // 2, c % 2
        cs = slice(hg * HPC * D, (hg + 1) * HPC * D)
        in_maps.append({
            "XQ": q16[b], "XK": k16[b], "XV": v16[b],
            "M8": m8[b],
            "WQ": np.ascontiguousarray(Wq16[:, cs]),
            "WK": np.ascontiguousarray(Wk16[:, cs]),
            "WV": np.ascontiguousarray(Wv16[:, cs]),
            "WO": np.ascontiguousarray(Wo16[cs, :]),
            "BQ": np.ascontiguousarray(bq16[:, cs]),
            "BK": np.ascontiguousarray(bk16[:, cs]),
            "BV": np.ascontiguousarray(bv16[:, cs]),
        })
    return in_maps


def postprocess(res, bo):
    out = np.empty((B, S, D), np.float32)
    attn = np.empty((H, B, S, S), np.float32)
    for c in range(8):
        b, hg = c // 2, c % 2
        r = res.results[c]
        attn[hg * HPC:(hg + 1) * HPC, b] = r["ATTN"].astype(np.float32)
        if hg == 0:
            out[b] = r["YT"].T
        else:
            out[b] += r["YT"].T
    out += np.asarray(bo).astype(np.float32)
    return out, attn


def kernel(query, keys, values, pad_mask, look_ahead_mask,
           Wq, bq, Wk, bk, Wv, bv, Wo, bo):
    in_maps = prepare_in_maps(query, keys, values, pad_mask, look_ahead_mask,
                              Wq, bq, Wk, bk, Wv, bv, Wo, bo)
    nc = _get_nc()
    res = bass_utils.run_bass_kernel_spmd(nc, in_maps, list(range(8)))
    return postprocess(res, bo)


# revision 9
# speedup vs baseline: 2.3776x; 1.0859x over previous
"""Multi-head attention TRN2 Bass kernel (B=4, S=1024, D=512, H=8, per-head dim 512).

Sharding: 8 cores = (batch, head-group) grid: core c handles batch c//2 and
heads (c%2)*4 .. (c%2)*4+3. Each core computes q/k/v projections for its
heads (feature-major via DMA-transposed x), masked softmax (fp32, exact
reference semantics incl. fully-masked rows), the attention output slice
(bf16, host casts to f32), and a partial y.T = Wo_slice.T @ ctx.T which the
host sums across the two head-group cores per batch. All matmuls bf16 with
fp32 PSUM accumulation; softmax math fp32. attn.T for the A@V matmul comes
from big [S,128]->[128,S] DMA xbar transposes reading back the attn DRAM
output (shadow-memory RAW deps order the readback after the writes).
"""

import numpy as np
import ml_dtypes

import concourse.bass as bass
import concourse.mybir as mybir
import concourse.tile as tile
from concourse import bacc, bass_utils
from concourse.alu_op_type import AluOpType

F32 = mybir.dt.float32
BF16 = mybir.dt.bfloat16
I8 = mybir.dt.int8

B, S, D, H = 4, 1024, 512, 8
HPC = 4                      # heads per core
NEG = float(np.float32(-1.0e15))
S_SCALE = float(1.0 / np.sqrt(np.float32(D)))

_CACHE = {}


def build(with_bias=True):
    nc = bacc.Bacc("TRN2", target_bir_lowering=False, debug=False, num_devices=8)

    XQ = nc.dram_tensor("XQ", [S, D], BF16, kind="ExternalInput").ap()
    XK = nc.dram_tensor("XK", [S, D], BF16, kind="ExternalInput").ap()
    XV = nc.dram_tensor("XV", [S, D], BF16, kind="ExternalInput").ap()
    M8 = nc.dram_tensor("M8", [S, S], I8, kind="ExternalInput").ap()
    WQ = nc.dram_tensor("WQ", [D, HPC * D], BF16, kind="ExternalInput").ap()
    WK = nc.dram_tensor("WK", [D, HPC * D], BF16, kind="ExternalInput").ap()
    WV = nc.dram_tensor("WV", [D, HPC * D], BF16, kind="ExternalInput").ap()
    WO = nc.dram_tensor("WO", [HPC * D, D], BF16, kind="ExternalInput").ap()
    if with_bias:
        BQ = nc.dram_tensor("BQ", [1, HPC * D], BF16, kind="ExternalInput").ap()
        BK = nc.dram_tensor("BK", [1, HPC * D], BF16, kind="ExternalInput").ap()
        BV = nc.dram_tensor("BV", [1, HPC * D], BF16, kind="ExternalInput").ap()

    ATTN = nc.dram_tensor("ATTN", [HPC, S, S], BF16, kind="ExternalOutput").ap()
    YT = nc.dram_tensor("YT", [D, S], F32, kind="ExternalOutput").ap()

    NT = S // 128            # 8 token tiles
    NF = D // 128            # 4 feature chunks

    with tile.TileContext(nc) as tc:
        with (
            tc.tile_pool(name="pc", bufs=1) as pc,
            tc.tile_pool(name="px", bufs=1) as px,
            tc.tile_pool(name="pw", bufs=2) as pw,
            tc.tile_pool(name="pqkv", bufs=2) as pqkv,
            tc.tile_pool(name="psm", bufs=2) as psm,
            tc.tile_pool(name="psmall", bufs=6) as psmall,
            tc.tile_pool(name="pet", bufs=2) as pet,
            tc.tile_pool(name="pps_proj", bufs=2, space="PSUM") as pps_proj,
            tc.tile_pool(name="pps_log", bufs=2, space="PSUM") as pps_log,
            tc.tile_pool(name="pps_ctx", bufs=2, space="PSUM") as pps_ctx,
            tc.tile_pool(name="pps_y", bufs=2, space="PSUM") as pps_y,
        ):
            ones512 = pc.tile([1, 512], BF16, tag="ones512")
            nc.vector.memset(ones512[:], 1.0)

            # ---- feature-major x.T via big DRAM->SBUF xbar transposes ----
            xqT = [px.tile([128, S], BF16, tag=f"xqT{f}", name=f"xqT{f}") for f in range(NF)]
            xkT = [px.tile([128, S], BF16, tag=f"xkT{f}", name=f"xkT{f}") for f in range(NF)]
            xvT = [px.tile([128, S], BF16, tag=f"xvT{f}", name=f"xvT{f}") for f in range(NF)]
            for xin, xT in ((XQ, xqT), (XK, xkT), (XV, xvT)):
                for f in range(NF):
                    nc.sync.dma_start(xT[f][:], xin[:, f * 128:(f + 1) * 128],
                                      transpose=True)

            # ---- masks (int8, 0/1/2), one [128,S] tile per sq-tile ----
            msk = [px.tile([128, S], I8, tag=f"m{i}", name=f"m{i}") for i in range(NT)]
            for i in range(NT):
                nc.sync.dma_start(msk[i][:], M8[i * 128:(i + 1) * 128, :])

            # ---- y.T accumulator (fp32) ----
            yacc = [px.tile([128, S], F32, tag=f"yacc{m}", name=f"yacc{m}") for m in range(NF)]
            for m in range(NF):
                nc.vector.memset(yacc[m][:], 0.0)

            for h in range(HPC):
                hs = h * D

                # ---- per-head weights, one batched 3D-AP DMA each ----
                wq = pw.tile([128, NF, D], BF16, tag="wq", name=f"wq_h{h}")
                wk = pw.tile([128, NF, D], BF16, tag="wk", name=f"wk_h{h}")
                wv = pw.tile([128, NF, D], BF16, tag="wv", name=f"wv_h{h}")
                wo = pw.tile([128, NF, D], BF16, tag="wo", name=f"wo_h{h}")
                nc.sync.dma_start(
                    wq[:], WQ[:, hs:hs + D].rearrange("(k p) m -> p k m", p=128))
                nc.sync.dma_start(
                    wk[:], WK[:, hs:hs + D].rearrange("(k p) m -> p k m", p=128))
                nc.sync.dma_start(
                    wv[:], WV[:, hs:hs + D].rearrange("(k p) m -> p k m", p=128))
                nc.sync.dma_start(
                    wo[:], WO[hs:hs + D, :].rearrange("(k p) m -> p k m", p=128))
                if with_bias:
                    bq = pw.tile([1, D], BF16, tag="bq", name=f"bq_h{h}")
                    nc.sync.dma_start(bq[:], BQ[:, hs:hs + D])
                    bk = pw.tile([1, D], BF16, tag="bk", name=f"bk_h{h}")
                    nc.sync.dma_start(bk[:], BK[:, hs:hs + D])
                    bv = pw.tile([1, D], BF16, tag="bv", name=f"bv_h{h}")
                    nc.sync.dma_start(bv[:], BV[:, hs:hs + D])
                else:
                    bq = bk = bv = None

                # ---- projections ----
                # q,k feature-major: [feat-tile 128, tokens]; q pre-scaled by 1/sqrt(D)
                qT = [pqkv.tile([128, S], BF16, tag=f"qT{m}", name=f"qT{m}_h{h}") for m in range(NF)]
                kT = [pqkv.tile([128, S], BF16, tag=f"kT{m}", name=f"kT{m}_h{h}") for m in range(NF)]
                for dst, w, b, xT, scale, use_act in (
                    (qT, wq, bq, xqT, S_SCALE, True),
                    (kT, wk, bk, xkT, 1.0, False),
                ):
                    for m in range(NF):
                        for n in range(S // 512):
                            ps = pps_proj.tile([128, 512], F32, tag="proj")
                            for kc in range(NF):
                                nc.tensor.matmul(
                                    ps[:], w[:, kc, m * 128:(m + 1) * 128],
                                    xT[kc][:, n * 512:(n + 1) * 512],
                                    start=(kc == 0), stop=(not with_bias and kc == NF - 1))
                            if with_bias:
                                nc.tensor.matmul(
                                    ps[:], b[:, m * 128:(m + 1) * 128], ones512[:],
                                    start=False, stop=True)
                            if use_act:
                                nc.scalar.activation(
                                    dst[m][:, n * 512:(n + 1) * 512], ps[:],
                                    mybir.ActivationFunctionType.Copy, scale=scale)
                            else:
                                nc.vector.tensor_copy(
                                    dst[m][:, n * 512:(n + 1) * 512], ps[:])

                # ---- attention: logits -> softmax -> attn out (bf16) ----
                for i in range(NT):
                    ps_pair = []
                    for nk in range(S // 512):
                        ps = pps_log.tile([128, 512], F32, tag="log")
                        for kc in range(NF):
                            nc.tensor.matmul(
                                ps[:], qT[kc][:, i * 128:(i + 1) * 128],
                                kT[kc][:, nk * 512:(nk + 1) * 512],
                                start=(kc == 0), stop=(kc == NF - 1))
                        ps_pair.append(ps)
                    X = psm.tile([128, S], F32, tag="X")
                    for nk in range(S // 512):
                        nc.vector.scalar_tensor_tensor(
                            X[:, nk * 512:(nk + 1) * 512],
                            msk[i][:, nk * 512:(nk + 1) * 512], NEG,
                            ps_pair[nk][:], op0=AluOpType.mult, op1=AluOpType.add)
                    negmax = psmall.tile([128, 1], F32, tag="negmax")
                    nc.vector.tensor_reduce(
                        negmax[:], X[:], axis=mybir.AxisListType.X,
                        op=AluOpType.max, negate=True)
                    E = psm.tile([128, S], F32, tag="E")
                    rowsum = psmall.tile([128, 1], F32, tag="rowsum")
                    nc.scalar.activation(
                        E[:], X[:], mybir.ActivationFunctionType.Exp,
                        bias=negmax[:], accum_out=rowsum[:])
                    recip = psmall.tile([128, 1], F32, tag="recip")
                    nc.vector.reciprocal(recip[:], rowsum[:])
                    attnb = psm.tile([128, S], BF16, tag="attnb")
                    nc.scalar.activation(
                        attnb[:], E[:], mybir.ActivationFunctionType.Copy,
                        scale=recip[:])
                    nc.sync.dma_start(ATTN[h, i * 128:(i + 1) * 128, :], attnb[:])

                # v token-major: [tok-tile 128, dv 512]
                v_sb = [pqkv.tile([128, D], BF16, tag=f"v{t}", name=f"v{t}_h{h}") for t in range(NT)]
                for t in range(NT):
                    ps = pps_proj.tile([128, 512], F32, tag="proj")
                    for kc in range(NF):
                        nc.tensor.matmul(
                            ps[:], xvT[kc][:, t * 128:(t + 1) * 128], wv[:, kc, :],
                            start=(kc == 0), stop=(not with_bias and kc == NF - 1))
                    if with_bias:
                        nc.tensor.matmul(ps[:], ones512[:, 0:128], bv[:],
                                         start=False, stop=True)
                    nc.vector.tensor_copy(v_sb[t][:], ps[:])

                # ---- attn.T tiles via big DRAM->SBUF xbar transposes ----
                ET = [pet.tile([128, S], BF16, tag=f"ET{j}", name=f"ET{j}_h{h}") for j in range(NT)]
                for j in range(NT):
                    nc.sync.dma_start(ET[j][:], ATTN[h, :, j * 128:(j + 1) * 128],
                                      transpose=True)

                # ---- ctx.T = v.T @ attn.T : [dv-tile 128, sq] ----
                ctxT = [pqkv.tile([128, S], BF16, tag=f"ctxT{m}", name=f"ctxT{m}_h{h}") for m in range(NF)]
                for m in range(NF):
                    for n in range(S // 512):
                        ps = pps_ctx.tile([128, 512], F32, tag="ctx")
                        for skc in range(NT):
                            nc.tensor.matmul(
                                ps[:], v_sb[skc][:, m * 128:(m + 1) * 128],
                                ET[skc][:, n * 512:(n + 1) * 512],
                                start=(skc == 0), stop=(skc == NT - 1))
                        nc.scalar.activation(
                            ctxT[m][:, n * 512:(n + 1) * 512], ps[:],
                            mybir.ActivationFunctionType.Copy)

                # ---- partial out-proj: yacc += Wo_head chunks @ ctx.T ----
                for m in range(NF):
                    for n in range(S // 512):
                        ps = pps_y.tile([128, 512], F32, tag="y")
                        for kc in range(NF):
                            nc.tensor.matmul(
                                ps[:], wo[:, kc, m * 128:(m + 1) * 128],
                                ctxT[kc][:, n * 512:(n + 1) * 512],
                                start=(kc == 0), stop=(kc == NF - 1))
                        nc.vector.tensor_tensor(
                            yacc[m][:, n * 512:(n + 1) * 512],
                            yacc[m][:, n * 512:(n + 1) * 512], ps[:],
                            op=AluOpType.add)

            for m in range(NF):
                nc.sync.dma_start(YT[m * 128:(m + 1) * 128, :], yacc[m][:])

    nc.compile()
    return nc


def _get_nc(with_bias=True):
    key = f"nc{int(with_bias)}"
    if key not in _CACHE:
        _CACHE[key] = build(with_bias)
    return _CACHE[key]


def prepare_in_maps(query, keys, values, pad_mask, look_ahead_mask,
                    Wq, bq, Wk, bk, Wv, bv, Wo, bo, with_bias=True):
    query = np.asarray(query)
    keys = np.asarray(keys)
    values = np.asarray(values)
    m8 = (np.asarray(pad_mask) + np.asarray(look_ahead_mask)).astype(np.int8)

    bf = ml_dtypes.bfloat16
    q16 = query.astype(bf)
    k16 = keys.astype(bf)
    v16 = values.astype(bf)
    Wq16 = np.asarray(Wq).astype(bf)
    Wk16 = np.asarray(Wk).astype(bf)
    Wv16 = np.asarray(Wv).astype(bf)
    Wo16 = np.asarray(Wo).astype(bf)
    bq16 = np.asarray(bq).astype(bf).reshape(1, -1)
    bk16 = np.asarray(bk).astype(bf).reshape(1, -1)
    bv16 = np.asarray(bv).astype(bf).reshape(1, -1)

    in_maps = []
    for c in range(8):
        b, hg = c // 2, c % 2
        cs = slice(hg * HPC * D, (hg + 1) * HPC * D)
        in_maps.append({
            "XQ": q16[b], "XK": k16[b], "XV": v16[b],
            "M8": m8[b],
            "WQ": np.ascontiguousarray(Wq16[:, cs]),
            "WK": np.ascontiguousarray(Wk16[:, cs]),
            "WV": np.ascontiguousarray(Wv16[:, cs]),
            "WO": np.ascontiguousarray(Wo16[cs, :]),
            **({"BQ": np.ascontiguousarray(bq16[:, cs]),
                "BK": np.ascontiguousarray(bk16[:, cs]),
                "BV": np.ascontiguousarray(bv16[:, cs])} if with_bias else {}),
        })
    return in_maps


def postprocess(res, bo):
    out = np.empty((B, S, D), np.float32)
    attn = np.empty((H, B, S, S), np.float32)
    for c in range(8):
        b, hg = c // 2, c % 2
        r = res.results[c]
        attn[hg * HPC:(hg + 1) * HPC, b] = r["ATTN"].astype(np.float32)
        if hg == 0:
            out[b] = r["YT"].T
        else:
            out[b] += r["YT"].T
    out += np.asarray(bo).astype(np.float32)
    return out, attn


def kernel(query, keys, values, pad_mask, look_ahead_mask,
           Wq, bq, Wk, bk, Wv, bv, Wo, bo):
    with_bias = bool(np.any(np.asarray(bq)) or np.any(np.asarray(bk))
                     or np.any(np.asarray(bv)))
    in_maps = prepare_in_maps(query, keys, values, pad_mask, look_ahead_mask,
                              Wq, bq, Wk, bk, Wv, bv, Wo, bo, with_bias)
    nc = _get_nc(with_bias)
    res = bass_utils.run_bass_kernel_spmd(nc, in_maps, list(range(8)))
    return postprocess(res, bo)


# revision 11
# speedup vs baseline: 2.4919x; 1.0481x over previous
"""Multi-head attention TRN2 Bass kernel (B=4, S=1024, D=512, H=8, per-head dim 512).

Sharding: 8 cores = (batch, head-group) grid: core c handles batch c//2 and
heads (c%2)*4 .. (c%2)*4+3. Each core computes q/k/v projections for its
heads (feature-major via DMA-transposed x), masked softmax (fp32, exact
reference semantics incl. fully-masked rows), the attention output slice
(bf16, host casts to f32), and a partial y.T = Wo_slice.T @ ctx.T which the
host sums across the two head-group cores per batch. All matmuls bf16 with
fp32 PSUM accumulation; softmax math fp32. attn.T for the A@V matmul comes
from big [S,128]->[128,S] DMA xbar transposes reading back the attn DRAM
output (shadow-memory RAW deps order the readback after the writes).
"""

import numpy as np
import ml_dtypes

import concourse.bass as bass
import concourse.mybir as mybir
import concourse.tile as tile
from concourse import bacc, bass_utils
from concourse.alu_op_type import AluOpType

F32 = mybir.dt.float32
BF16 = mybir.dt.bfloat16
I8 = mybir.dt.int8

B, S, D, H = 4, 1024, 512, 8
HPC = 4                      # heads per core
NEG = float(np.float32(-1.0e15))
S_SCALE = float(1.0 / np.sqrt(np.float32(D)))

_CACHE = {}


def build(with_bias=True, causal=True):
    nc = bacc.Bacc("TRN2", target_bir_lowering=False, debug=False, num_devices=8)

    XQ = nc.dram_tensor("XQ", [S, D], BF16, kind="ExternalInput").ap()
    XK = nc.dram_tensor("XK", [S, D], BF16, kind="ExternalInput").ap()
    XV = nc.dram_tensor("XV", [S, D], BF16, kind="ExternalInput").ap()
    M8 = nc.dram_tensor("M8", [S, S], I8, kind="ExternalInput").ap()
    WQ = nc.dram_tensor("WQ", [D, HPC * D], BF16, kind="ExternalInput").ap()
    WK = nc.dram_tensor("WK", [D, HPC * D], BF16, kind="ExternalInput").ap()
    WV = nc.dram_tensor("WV", [D, HPC * D], BF16, kind="ExternalInput").ap()
    WO = nc.dram_tensor("WO", [HPC * D, D], BF16, kind="ExternalInput").ap()
    if with_bias:
        BQ = nc.dram_tensor("BQ", [1, HPC * D], BF16, kind="ExternalInput").ap()
        BK = nc.dram_tensor("BK", [1, HPC * D], BF16, kind="ExternalInput").ap()
        BV = nc.dram_tensor("BV", [1, HPC * D], BF16, kind="ExternalInput").ap()

    ATTN = nc.dram_tensor("ATTN", [HPC, S, S], BF16, kind="ExternalOutput").ap()
    YT = nc.dram_tensor("YT", [D, S], F32, kind="ExternalOutput").ap()

    NT = S // 128            # 8 token tiles
    NF = D // 128            # 4 feature chunks

    with tile.TileContext(nc) as tc:
        with (
            tc.tile_pool(name="pc", bufs=1) as pc,
            tc.tile_pool(name="px", bufs=1) as px,
            tc.tile_pool(name="pw", bufs=2) as pw,
            tc.tile_pool(name="pqkv", bufs=2) as pqkv,
            tc.tile_pool(name="psm", bufs=2) as psm,
            tc.tile_pool(name="psmall", bufs=6) as psmall,
            tc.tile_pool(name="pet", bufs=2) as pet,
            tc.tile_pool(name="pps_proj", bufs=2, space="PSUM") as pps_proj,
            tc.tile_pool(name="pps_log", bufs=3, space="PSUM") as pps_log,
            tc.tile_pool(name="pps_ctx", bufs=2, space="PSUM") as pps_ctx,
            tc.tile_pool(name="pps_y", bufs=1, space="PSUM") as pps_y,
        ):
            ones512 = pc.tile([1, 512], BF16, tag="ones512")
            nc.vector.memset(ones512[:], 1.0)

            # ---- feature-major x.T via big DRAM->SBUF xbar transposes ----
            xqT = [px.tile([128, S], BF16, tag=f"xqT{f}", name=f"xqT{f}") for f in range(NF)]
            xkT = [px.tile([128, S], BF16, tag=f"xkT{f}", name=f"xkT{f}") for f in range(NF)]
            xvT = [px.tile([128, S], BF16, tag=f"xvT{f}", name=f"xvT{f}") for f in range(NF)]
            for xin, xT in ((XQ, xqT), (XK, xkT), (XV, xvT)):
                for f in range(NF):
                    nc.sync.dma_start(xT[f][:], xin[:, f * 128:(f + 1) * 128],
                                      transpose=True)

            # ---- masks (int8, 0/1/2), one [128,S] tile per sq-tile ----
            msk = [px.tile([128, S], I8, tag=f"m{i}", name=f"m{i}") for i in range(NT)]
            for i in range(NT):
                nc.sync.dma_start(msk[i][:], M8[i * 128:(i + 1) * 128, :])

            # ---- y.T accumulator (fp32) ----
            yacc = [px.tile([128, S], F32, tag=f"yacc{m}", name=f"yacc{m}") for m in range(NF)]
            for m in range(NF):
                nc.vector.memset(yacc[m][:], 0.0)

            for h in range(HPC):
                hs = h * D

                # ---- per-head weights, one batched 3D-AP DMA each ----
                wq = pw.tile([128, NF, D], BF16, tag="wq", name=f"wq_h{h}")
                wk = pw.tile([128, NF, D], BF16, tag="wk", name=f"wk_h{h}")
                wv = pw.tile([128, NF, D], BF16, tag="wv", name=f"wv_h{h}")
                wo = pw.tile([128, NF, D], BF16, tag="wo", name=f"wo_h{h}")
                nc.sync.dma_start(
                    wq[:], WQ[:, hs:hs + D].rearrange("(k p) m -> p k m", p=128))
                nc.sync.dma_start(
                    wk[:], WK[:, hs:hs + D].rearrange("(k p) m -> p k m", p=128))
                nc.sync.dma_start(
                    wv[:], WV[:, hs:hs + D].rearrange("(k p) m -> p k m", p=128))
                nc.sync.dma_start(
                    wo[:], WO[hs:hs + D, :].rearrange("(k p) m -> p k m", p=128))
                if with_bias:
                    bq = pw.tile([1, D], BF16, tag="bq", name=f"bq_h{h}")
                    nc.sync.dma_start(bq[:], BQ[:, hs:hs + D])
                    bk = pw.tile([1, D], BF16, tag="bk", name=f"bk_h{h}")
                    nc.sync.dma_start(bk[:], BK[:, hs:hs + D])
                    bv = pw.tile([1, D], BF16, tag="bv", name=f"bv_h{h}")
                    nc.sync.dma_start(bv[:], BV[:, hs:hs + D])
                else:
                    bq = bk = bv = None

                # ---- projections ----
                # q,k feature-major: [feat-tile 128, tokens]; q pre-scaled by 1/sqrt(D)
                qT = [pqkv.tile([128, S], BF16, tag=f"qT{m}", name=f"qT{m}_h{h}") for m in range(NF)]
                kT = [pqkv.tile([128, S], BF16, tag=f"kT{m}", name=f"kT{m}_h{h}") for m in range(NF)]
                for dst, w, b, xT, scale, use_act in (
                    (qT, wq, bq, xqT, S_SCALE, True),
                    (kT, wk, bk, xkT, 1.0, False),
                ):
                    for m in range(NF):
                        for n in range(S // 512):
                            ps = pps_proj.tile([128, 512], F32, tag="proj")
                            for kc in range(NF):
                                nc.tensor.matmul(
                                    ps[:], w[:, kc, m * 128:(m + 1) * 128],
                                    xT[kc][:, n * 512:(n + 1) * 512],
                                    start=(kc == 0), stop=(not with_bias and kc == NF - 1))
                            if with_bias:
                                nc.tensor.matmul(
                                    ps[:], b[:, m * 128:(m + 1) * 128], ones512[:],
                                    start=False, stop=True)
                            if use_act:
                                nc.scalar.activation(
                                    dst[m][:, n * 512:(n + 1) * 512], ps[:],
                                    mybir.ActivationFunctionType.Copy, scale=scale)
                            else:
                                nc.vector.tensor_copy(
                                    dst[m][:, n * 512:(n + 1) * 512], ps[:])

                # ---- attention: logits -> softmax -> attn out (bf16) ----
                for i in range(NT):
                    # causal: sk columns >= (i+1)*128 are fully masked for
                    # this sq-tile; with 512-wide chunks, chunk nk is live
                    # iff nk*512 <= i*128 + 127
                    nlive = S // 512 if not causal else (i * 128) // 512 + 1
                    W = nlive * 512
                    ps_pair = []
                    for nk in range(nlive):
                        ps = pps_log.tile([128, 512], F32, tag="log")
                        for kc in range(NF):
                            nc.tensor.matmul(
                                ps[:], qT[kc][:, i * 128:(i + 1) * 128],
                                kT[kc][:, nk * 512:(nk + 1) * 512],
                                start=(kc == 0), stop=(kc == NF - 1))
                        ps_pair.append(ps)
                    X = psm.tile([128, S], F32, tag="X")
                    for nk in range(nlive):
                        nc.vector.scalar_tensor_tensor(
                            X[:, nk * 512:(nk + 1) * 512],
                            msk[i][:, nk * 512:(nk + 1) * 512], NEG,
                            ps_pair[nk][:], op0=AluOpType.mult, op1=AluOpType.add)
                    negmax = psmall.tile([128, 1], F32, tag="negmax")
                    nc.vector.tensor_reduce(
                        negmax[:], X[:, 0:W], axis=mybir.AxisListType.X,
                        op=AluOpType.max, negate=True)
                    E = psm.tile([128, S], F32, tag="E")
                    rowsum = psmall.tile([128, 1], F32, tag="rowsum")
                    nc.scalar.activation(
                        E[:, 0:W], X[:, 0:W], mybir.ActivationFunctionType.Exp,
                        bias=negmax[:], accum_out=rowsum[:])
                    recip = psmall.tile([128, 1], F32, tag="recip")
                    nc.vector.reciprocal(recip[:], rowsum[:])
                    attnb = psm.tile([128, S], BF16, tag="attnb")
                    nc.scalar.activation(
                        attnb[:, 0:W], E[:, 0:W],
                        mybir.ActivationFunctionType.Copy,
                        scale=recip[:])
                    if W < S:
                        nc.vector.memset(attnb[:, W:S], 0.0)
                    nc.sync.dma_start(ATTN[h, i * 128:(i + 1) * 128, :], attnb[:])

                # v token-major: [tok-tile 128, dv 512]
                v_sb = [pqkv.tile([128, D], BF16, tag=f"v{t}", name=f"v{t}_h{h}") for t in range(NT)]
                for t in range(NT):
                    ps = pps_proj.tile([128, 512], F32, tag="proj")
                    for kc in range(NF):
                        nc.tensor.matmul(
                            ps[:], xvT[kc][:, t * 128:(t + 1) * 128], wv[:, kc, :],
                            start=(kc == 0), stop=(not with_bias and kc == NF - 1))
                    if with_bias:
                        nc.tensor.matmul(ps[:], ones512[:, 0:128], bv[:],
                                         start=False, stop=True)
                    nc.vector.tensor_copy(v_sb[t][:], ps[:])

                # ---- attn.T tiles via big DRAM->SBUF xbar transposes ----
                ET = [pet.tile([128, S], BF16, tag=f"ET{j}", name=f"ET{j}_h{h}") for j in range(NT)]
                for j in range(NT):
                    nc.sync.dma_start(ET[j][:], ATTN[h, :, j * 128:(j + 1) * 128],
                                      transpose=True)

                # ---- ctx.T = v.T @ attn.T : [dv-tile 128, sq] ----
                ctxT = [pqkv.tile([128, S], BF16, tag=f"ctxT{m}", name=f"ctxT{m}_h{h}") for m in range(NF)]
                for m in range(NF):
                    for n in range(S // 512):
                        # causal: sk-tile skc contributes to sq-chunk n only
                        # if skc*128 <= n*512 + 511
                        nsk = NT if not causal else min(NT, (n * 512 + 511) // 128 + 1)
                        ps = pps_ctx.tile([128, 512], F32, tag="ctx")
                        for skc in range(nsk):
                            nc.tensor.matmul(
                                ps[:], v_sb[skc][:, m * 128:(m + 1) * 128],
                                ET[skc][:, n * 512:(n + 1) * 512],
                                start=(skc == 0), stop=(skc == nsk - 1))
                        nc.scalar.activation(
                            ctxT[m][:, n * 512:(n + 1) * 512], ps[:],
                            mybir.ActivationFunctionType.Copy)

                # ---- partial out-proj: yacc += Wo_head chunks @ ctx.T ----
                for m in range(NF):
                    for n in range(S // 512):
                        ps = pps_y.tile([128, 512], F32, tag="y")
                        for kc in range(NF):
                            nc.tensor.matmul(
                                ps[:], wo[:, kc, m * 128:(m + 1) * 128],
                                ctxT[kc][:, n * 512:(n + 1) * 512],
                                start=(kc == 0), stop=(kc == NF - 1))
                        nc.vector.tensor_tensor(
                            yacc[m][:, n * 512:(n + 1) * 512],
                            yacc[m][:, n * 512:(n + 1) * 512], ps[:],
                            op=AluOpType.add)

            for m in range(NF):
                nc.sync.dma_start(YT[m * 128:(m + 1) * 128, :], yacc[m][:])

    nc.compile()
    return nc


def _get_nc(with_bias=True, causal=True):
    key = f"nc{int(with_bias)}{int(causal)}"
    if key not in _CACHE:
        _CACHE[key] = build(with_bias, causal)
    return _CACHE[key]


def prepare_in_maps(query, keys, values, pad_mask, look_ahead_mask,
                    Wq, bq, Wk, bk, Wv, bv, Wo, bo, with_bias=True):
    query = np.asarray(query)
    keys = np.asarray(keys)
    values = np.asarray(values)
    m8 = (np.asarray(pad_mask) + np.asarray(look_ahead_mask)).astype(np.int8)

    bf = ml_dtypes.bfloat16
    q16 = query.astype(bf)
    k16 = keys.astype(bf)
    v16 = values.astype(bf)
    Wq16 = np.asarray(Wq).astype(bf)
    Wk16 = np.asarray(Wk).astype(bf)
    Wv16 = np.asarray(Wv).astype(bf)
    Wo16 = np.asarray(Wo).astype(bf)
    bq16 = np.asarray(bq).astype(bf).reshape(1, -1)
    bk16 = np.asarray(bk).astype(bf).reshape(1, -1)
    bv16 = np.asarray(bv).astype(bf).reshape(1, -1)

    in_maps = []
    for c in range(8):
        b, hg = c // 2, c % 2
        cs = slice(hg * HPC * D, (hg + 1) * HPC * D)
        in_maps.append({
            "XQ": q16[b], "XK": k16[b], "XV": v16[b],
            "M8": m8[b],
            "WQ": np.ascontiguousarray(Wq16[:, cs]),
            "WK": np.ascontiguousarray(Wk16[:, cs]),
            "WV": np.ascontiguousarray(Wv16[:, cs]),
            "WO": np.ascontiguousarray(Wo16[cs, :]),
            **({"BQ": np.ascontiguousarray(bq16[:, cs]),
                "BK": np.ascontiguousarray(bk16[:, cs]),
                "BV": np.ascontiguousarray(bv16[:, cs])} if with_bias else {}),
        })
    return in_maps


def postprocess(res, bo):
    out = np.empty((B, S, D), np.float32)
    attn = np.empty((H, B, S, S), np.float32)
    for c in range(8):
        b, hg = c // 2, c % 2
        r = res.results[c]
        attn[hg * HPC:(hg + 1) * HPC, b] = r["ATTN"].astype(np.float32)
        if hg == 0:
            out[b] = r["YT"].T
        else:
            out[b] += r["YT"].T
    out += np.asarray(bo).astype(np.float32)
    return out, attn


def _degenerate_fixup(out, attn, m, values, Wv, bv, Wo, bo):
    """Exact handling of fully-masked query rows: the reference's softmax
    spreads attention uniformly over the least-masked positions (which can
    include causally-masked ones), so recompute those rows on the host."""
    deg = (m >= 1).all(axis=2)
    if not deg.any():
        return
    Wv = np.asarray(Wv, np.float32)
    bv = np.asarray(bv, np.float32)
    Wo = np.asarray(Wo, np.float32)
    bo = np.asarray(bo, np.float32)
    for b, q in zip(*np.where(deg)):
        row = m[b, q]
        lvl = row.min()
        sel = row == lvl
        cnt = int(sel.sum())
        aval = np.float32(1.0) / np.float32(cnt)
        attn[:, b, q, :] = np.where(sel, aval, np.float32(0.0))
        xmean = (values[b][sel].sum(axis=0, dtype=np.float64) / cnt).astype(np.float32)
        vrow = xmean @ Wv + bv
        out[b, q] = vrow.reshape(H, D).reshape(-1) @ Wo + bo


def kernel(query, keys, values, pad_mask, look_ahead_mask,
           Wq, bq, Wk, bk, Wv, bv, Wo, bo):
    with_bias = bool(np.any(np.asarray(bq)) or np.any(np.asarray(bk))
                     or np.any(np.asarray(bv)))
    la = np.asarray(look_ahead_mask)
    causal = bool((la == np.broadcast_to(
        np.triu(np.ones((S, S), la.dtype), k=1), la.shape)).all())
    in_maps = prepare_in_maps(query, keys, values, pad_mask, look_ahead_mask,
                              Wq, bq, Wk, bk, Wv, bv, Wo, bo, with_bias)
    nc = _get_nc(with_bias, causal)
    res = bass_utils.run_bass_kernel_spmd(nc, in_maps, list(range(8)))
    out, attn = postprocess(res, bo)
    m = (np.asarray(pad_mask) + np.asarray(look_ahead_mask))
    _degenerate_fixup(out, attn, m, np.asarray(values, np.float32),
                      Wv, bv, Wo, bo)
    return out, attn
